# revision 4
# baseline (speedup 1.0000x reference)
"""Mamba2-style chunked SSD scan on 8 Trainium2 NeuronCores.

Full-input contract: kernel(X, A, B, C, initial_states) -> Y, with
  X: (b, s, h, p) f32   A: (b, s, h) f32   B, C: (b, s, h, n) f32
  initial_states: (b, 1, h, p, n) f32      Y: (b, s, h, p) f32

Sharding: heads across the 8 cores (h % 8 == 0); every core runs an
identical program over its own (b, h/8) slice -- no collectives.

Per-core algorithm (block length L=128, c = s/L chunks per (b,h) stream):
  cum      = within-chunk inclusive cumsum of A          (host, tiny)
  G^T      = B @ C^T           contraction over n        (PE, needs B^T/C^T
                                                          from one PE transpose)
  M~[s,i]  = G^T[s,i] * exp(-cum_s) * mask(s<=i)         (ACT scale + DVE mask)
  Y[i,:]   = exp(cum_i) * ( M~^T @ X  +  C @ R )         (2 matmuls, one PSUM)
  S        = (exp(cum_L - cum) * B)^T @ X                (PE)
  R       <- exp(cum_L) * R + S                          (DVE, fused)

All decay factors enter as per-partition scale columns (host-precomputed
exp tables, 4 columns per chunk), so no 128x128 decay matrix is built.
Matmul operands are bf16 (host-cast; f32 PSUM accumulation), state and
output stay f32.
"""

import os
from functools import lru_cache

import ml_dtypes
import numpy as np

L = 128  # chunk/block length (our choice; any block size is math-equivalent)
N_CORES = 8

_f32 = np.float32
_bf16 = ml_dtypes.bfloat16


def _maybe_enable_tracing():
    """Optional NTFF profiling (BASS_KERNEL_TRACE=1). Never required."""
    if not os.environ.get("BASS_KERNEL_TRACE"):
        return False
    try:
        import sys
        import types

        if "antenv.axon_hooks" not in sys.modules:
            mod = types.ModuleType("antenv.axon_hooks")
            mod._hook = None
            mod.set_axon_ntff_profile_hook = lambda h: setattr(mod, "_hook", h)
            mod.get_axon_ntff_profile_hook = lambda: mod._hook
            sys.modules["antenv.axon_hooks"] = mod
            from trn_agent_boot.trn_boot import _ntff_profile_via_ctypes

            hook = _ntff_profile_via_ctypes("/opt/axon/libaxon_pjrt.so")
            if hook is None:
                return False
            mod.set_axon_ntff_profile_hook(hook)
            import concourse.bass_utils as bu

            bu.upload_artifacts = lambda tmpdir: f"file://{tmpdir}"
        return True
    except Exception:
        return False


@lru_cache(maxsize=4)
def _build_program(b, s, hpc, p, n):
    """Build + compile the per-core Bass program.

    Per-core DRAM tensors:
      BCX  (b, s, hpc, 3*p)   bf16  [X | B | C] interleaved per (s, head)
      aux  (b, hpc, c, L, 4)  f32   exp(-cum), exp(cum), exp(tot-cum), exp(tot)
      init (b, hpc, n, p)     f32   initial state, (n, p) layout
      mask (L, L)             bf16  mask[s, i] = (s <= i)
      idn  (L, L)             bf16  identity for PE transpose
      Y    (b, s, hpc, p)     f32
    """
    import concourse.bacc as bacc
    import concourse.mybir as mybir
    import concourse.tile as tile

    dt = mybir.dt
    assert s % L == 0 and p == 64 and n == 64
    c = s // L
    nbh = b * hpc

    nc = bacc.Bacc("TRN2", target_bir_lowering=False, debug=False)

    bcx_d = nc.dram_tensor("BCX", [b, s, hpc, 3 * p], dt.bfloat16, kind="ExternalInput").ap()
    aux_d = nc.dram_tensor("aux", [b, hpc, c, L, 4], dt.float32, kind="ExternalInput").ap()
    init_d = nc.dram_tensor("init", [b, hpc, n, p], dt.float32, kind="ExternalInput").ap()
    mask_d = nc.dram_tensor("mask", [L, L], dt.bfloat16, kind="ExternalInput").ap()
    idn_d = nc.dram_tensor("idn", [L, L], dt.bfloat16, kind="ExternalInput").ap()
    y_d = nc.dram_tensor("Y", [b, s, hpc, p], dt.float32, kind="ExternalOutput").ap()

    bh = [(bi, hi) for bi in range(b) for hi in range(hpc)]

    with tile.TileContext(nc) as tc:
        with (
            tc.tile_pool(name="const", bufs=1) as cpool,
            tc.tile_pool(name="state", bufs=1) as rpool,
            tc.tile_pool(name="rb", bufs=2) as rbpool,
            tc.tile_pool(name="io", bufs=4) as iopool,
            tc.tile_pool(name="work", bufs=3) as wpool,
            tc.tile_pool(name="psT", bufs=2, space="PSUM") as psT,
            tc.tile_pool(name="psG", bufs=2, space="PSUM") as psG,
            tc.tile_pool(name="psY", bufs=2, space="PSUM") as psY,
            tc.tile_pool(name="psS", bufs=2, space="PSUM") as psS,
        ):
            mask_t = cpool.tile([L, L], dt.bfloat16, tag="mask")
            nc.sync.dma_start(mask_t[:], mask_d[:])
            idn_t = cpool.tile([L, L], dt.bfloat16, tag="idn")
            nc.sync.dma_start(idn_t[:], idn_d[:])

            r_ts = []
            rb_prev = []
            for i, (bi, hi) in enumerate(bh):
                r = rpool.tile([n, p], dt.float32, tag=f"R{i}")
                nc.sync.dma_start(r[:], init_d[bi, hi])
                r_ts.append(r)
                rb = rbpool.tile([n, p], dt.bfloat16, tag=f"rb{i}")
                nc.vector.tensor_copy(rb[:], r[:])
                rb_prev.append(rb)

            for ci in range(c):
                s0 = ci * L
                for i, (bi, hi) in enumerate(bh):
                    bcx = iopool.tile([L, 3 * p], dt.bfloat16, tag="bcx")
                    nc.sync.dma_start(bcx[:], bcx_d[bi, s0 : s0 + L, hi, :])
                    aux = iopool.tile([L, 4], dt.float32, tag="aux")
                    nc.sync.dma_start(aux[:], aux_d[bi, hi, ci])

                    x_sb = bcx[:, 0:p]

                    # [B | C] -> [B^T ; C^T] (n on partitions 0:64 / 64:128)
                    tps = psT.tile([L, L], dt.bfloat16, tag="tps")
                    nc.tensor.transpose(tps[:], bcx[:, p : 3 * p], idn_t[:])
                    tcb_b = wpool.tile([n, L], dt.bfloat16, tag="tcb_b")
                    nc.scalar.copy(tcb_b[:], tps[0:n, :])
                    tcb_c = wpool.tile([n, L], dt.bfloat16, tag="tcb_c")
                    nc.scalar.copy(tcb_c[:], tps[n : 2 * n, :])

                    # G^T[s,i] = sum_n B[s,n] C[i,n]
                    gps = psG.tile([L, L], dt.float32, tag="gps")
                    nc.tensor.matmul(
                        gps[:], tcb_b[:], tcb_c[:], start=True, stop=True
                    )

                    # M~ = G^T * exp(-cum_s) * mask(s<=i)
                    m1 = wpool.tile([L, L], dt.bfloat16, tag="m1")
                    nc.scalar.mul(m1[:], gps[:], aux[:, 0:1])
                    nc.vector.tensor_mul(m1[:], m1[:], mask_t[:])

                    # Y_raw = M~^T @ X + C @ R_prev ; Y = exp(cum_i) * Y_raw
                    yps = psY.tile([L, p], dt.float32, tag="yps")
                    nc.tensor.matmul(yps[:], m1[:], x_sb, start=True, stop=False)
                    nc.tensor.matmul(
                        yps[:], tcb_c[:], rb_prev[i][:], start=False, stop=True
                    )
                    yout = wpool.tile([L, p], dt.float32, tag="yout")
                    nc.scalar.mul(yout[:], yps[:], aux[:, 1:2])
                    nc.sync.dma_start(y_d[bi, s0 : s0 + L, hi, :], yout[:])

                    # S = (exp(tot - cum) * B)^T @ X
                    bd = wpool.tile([L, n], dt.bfloat16, tag="bd")
                    nc.scalar.mul(bd[:], bcx[:, p : 2 * p], aux[:, 2:3])
                    sps = psS.tile([n, p], dt.float32, tag="sps")
                    nc.tensor.matmul(sps[:], bd[:], x_sb, start=True, stop=True)

                    # R <- exp(tot) * R + S   (fused); re-cast for next chunk
                    r = r_ts[i]
                    nc.vector.scalar_tensor_tensor(
                        r[:],
                        r[:],
                        aux[0:n, 3:4],
                        sps[:],
                        op0=mybir.AluOpType.mult,
                        op1=mybir.AluOpType.add,
                    )
                    rb = rbpool.tile([n, p], dt.bfloat16, tag=f"rb{i}")
                    nc.vector.tensor_copy(rb[:], r[:])
                    rb_prev[i] = rb

    nc.compile()
    return nc


def kernel(X, A, B, C, initial_states):
    from concourse.bass_utils import run_bass_kernel_spmd

    X = np.asarray(X)
    A = np.asarray(A)
    B = np.asarray(B)
    C = np.asarray(C)
    initial_states = np.asarray(initial_states)

    b, s, h, p = X.shape
    n = B.shape[-1]
    assert h % N_CORES == 0, f"need h % {N_CORES} == 0, got h={h}"
    hpc = h // N_CORES
    c = s // L

    # ---- host prep (cheap, O(data) passes) ----
    # within-chunk inclusive cumsum of A, laid out (b, h, c, L)
    Ar = np.ascontiguousarray(A.reshape(b, c, L, h).transpose(0, 3, 1, 2))
    cum = np.cumsum(Ar, axis=-1, dtype=_f32)
    tot = cum[..., -1:]
    aux = np.stack(
        [
            np.exp(-cum),
            np.exp(cum),
            np.exp(tot - cum),
            np.broadcast_to(np.exp(tot), cum.shape),
        ],
        axis=-1,
    ).astype(_f32)  # (b, h, c, L, 4)

    bcx = np.concatenate([X, B, C], axis=-1).astype(_bf16)  # (b, s, h, 3p)
    init_t = np.ascontiguousarray(
        initial_states[:, 0].transpose(0, 1, 3, 2)
    ).astype(_f32)  # (b, h, n, p)

    mask = np.triu(np.ones((L, L), dtype=_f32)).astype(_bf16)  # mask[s,i] = s<=i
    idn = np.eye(L, dtype=_f32).astype(_bf16)

    nc = _build_program(b, s, hpc, p, n)

    in_maps = []
    for k in range(N_CORES):
        hs = slice(k * hpc, (k + 1) * hpc)
        in_maps.append(
            {
                "BCX": np.ascontiguousarray(bcx[:, :, hs, :]),
                "aux": np.ascontiguousarray(aux[:, hs]),
                "init": np.ascontiguousarray(init_t[:, hs]),
                "mask": mask,
                "idn": idn,
            }
        )

    trace = _maybe_enable_tracing()
    kw = {}
    if trace:
        kw = dict(trace=True, tmpdir=os.environ.get("BASS_KERNEL_TRACE_DIR") or None)
    res = run_bass_kernel_spmd(nc, in_maps, list(range(N_CORES)), **kw)
    if trace and res.exec_time_ns is not None:
        print(f"HW exec time: {res.exec_time_ns} ns")

    Y = np.concatenate([res.results[k]["Y"] for k in range(N_CORES)], axis=2)
    return np.ascontiguousarray(Y).astype(_f32)


# revision 5
# speedup vs baseline: 3.7484x; 3.7484x over previous
"""Mamba2-style chunked SSD scan on 8 Trainium2 NeuronCores.

Full-input contract: kernel(X, A, B, C, initial_states) -> Y, with
  X: (b, s, h, p) f32   A: (b, s, h) f32   B, C: (b, s, h, n) f32
  initial_states: (b, 1, h, p, n) f32      Y: (b, s, h, p) f32

Sharding: heads across the 8 cores (h % 8 == 0); every core runs an
identical program over its own (b, h/8) slice -- no collectives.

Math (block length L=128; c = s/L chunks per (b,h) stream; cum = within-
chunk inclusive cumsum of A):  with host-prescaled
    Bt[s,n] = B[s,n] * exp(-cum_s)      (bf16)
    Ct[i,n] = C[i,n] * exp(+cum_i)      (bf16)
the chunk output and state recurrence collapse to plain matmuls:
    Y[i,p] = sum_{s<=i} (Bt Ct^T)[s,i] X[s,p]  +  (Ct R)[i,p]
    R     <- exp(tot) * ( R + Bt^T X )
so the device does, per chunk (all 8 (b,h) bodies batched side by side):
    G   = Bt^T-slices @ Ct^T-slices      8 matmuls -> one PSUM tile
    M   = G * mask(s<=i)                 1 DVE op (bf16 out)
    Y   = M^T @ X + Ct @ R_prev          16 matmuls -> one PSUM tile
    S   = Bt^T @ X                       8 matmuls -> one PSUM tile
    R  += S; R *= exp(tot); rb = bf16(R) 1 DVE + 1 GpSimd + 1 ACT op
Host supplies Bt/Ct already transposed (n-major) so the kernel needs no
on-device transposes, and chunk PAIRS share single wide DMAs.
"""

import os
from functools import lru_cache

import ml_dtypes
import numpy as np

L = 128  # chunk/block length (our choice; any block size is math-equivalent)
N_CORES = 8

_f32 = np.float32
_bf16 = ml_dtypes.bfloat16

BODY_F = 132  # per-body column block in BCX: [X(64) | Bt(64) | aux(4)]


def _maybe_enable_tracing():
    """Optional NTFF profiling (BASS_KERNEL_TRACE=1). Never required."""
    if not os.environ.get("BASS_KERNEL_TRACE"):
        return False
    try:
        import sys
        import types

        if "antenv.axon_hooks" not in sys.modules:
            mod = types.ModuleType("antenv.axon_hooks")
            mod._hook = None
            mod.set_axon_ntff_profile_hook = lambda h: setattr(mod, "_hook", h)
            mod.get_axon_ntff_profile_hook = lambda: mod._hook
            sys.modules["antenv.axon_hooks"] = mod
            from trn_agent_boot.trn_boot import _ntff_profile_via_ctypes

            hook = _ntff_profile_via_ctypes("/opt/axon/libaxon_pjrt.so")
            if hook is None:
                return False
            mod.set_axon_ntff_profile_hook(hook)
            import concourse.bass_utils as bu

            bu.upload_artifacts = lambda tmpdir: f"file://{tmpdir}"
        return True
    except Exception:
        return False


@lru_cache(maxsize=4)
def _build_program(b, s, hpc, p, n):
    """Build + compile the per-core Bass program.

    Per-core DRAM tensors (c2 = chunk pairs, nbh = b*hpc bodies, F = nbh*BODY_F):
      BCX  (c2, L, 2*F)        bf16  [X | Bt | aux] per body, chunk pair in free
      TBT  (c2, n, 2*nbh*L)    bf16  Bt^T (n-major)
      TCT  (c2, n, 2*nbh*L)    bf16  Ct^T (n-major)
      INIT (n, nbh*p)          f32   initial states
      MASKX(L, nbh*L)          bf16  mask[s, i] = (s <= i), tiled per body
      Y    (c2, L, 2*nbh*p)    f32
    """
    import concourse.bacc as bacc
    import concourse.mybir as mybir
    import concourse.tile as tile

    dt = mybir.dt
    assert s % (2 * L) == 0 and p == 64 and n == 64
    c = s // L
    c2 = c // 2
    nbh = b * hpc
    F = nbh * BODY_F
    FB = nbh * L  # tbt/tct per-chunk free size
    FP = nbh * p  # y/s per-chunk free size

    nc = bacc.Bacc("TRN2", target_bir_lowering=False, debug=False)

    bcx_d = nc.dram_tensor("BCX", [c2, L, 2 * F], dt.bfloat16, kind="ExternalInput").ap()
    tbt_d = nc.dram_tensor("TBT", [c2, n, 2 * FB], dt.bfloat16, kind="ExternalInput").ap()
    tct_d = nc.dram_tensor("TCT", [c2, n, 2 * FB], dt.bfloat16, kind="ExternalInput").ap()
    init_d = nc.dram_tensor("INIT", [n, FP], dt.float32, kind="ExternalInput").ap()
    mask_d = nc.dram_tensor("MASKX", [L, FB], dt.bfloat16, kind="ExternalInput").ap()
    y_d = nc.dram_tensor("Y", [c2, L, 2 * FP], dt.float32, kind="ExternalOutput").ap()

    with tile.TileContext(nc) as tc:
        with (
            tc.tile_pool(name="const", bufs=1) as cpool,
            tc.tile_pool(name="state", bufs=1) as rpool,
            tc.tile_pool(name="rb", bufs=2) as rbpool,
            tc.tile_pool(name="io", bufs=3) as iopool,
            tc.tile_pool(name="tp", bufs=3) as tpool,
            tc.tile_pool(name="work", bufs=3) as wpool,
            tc.tile_pool(name="out", bufs=3) as opool,
            tc.tile_pool(name="psG", bufs=2, space="PSUM") as psG,
            tc.tile_pool(name="psY", bufs=2, space="PSUM") as psY,
            tc.tile_pool(name="psS", bufs=2, space="PSUM") as psS,
        ):
            maskx = cpool.tile([L, FB], dt.bfloat16, tag="maskx")
            nc.sync.dma_start(maskx[:], mask_d[:])

            r_big = rpool.tile([n, FP], dt.float32, tag="R")
            nc.sync.dma_start(r_big[:], init_d[:])
            rb_prev = rbpool.tile([n, FP], dt.bfloat16, tag="rb")
            nc.scalar.copy(rb_prev[:], r_big[:])

            for cp in range(c2):
                bcx2 = iopool.tile([L, 2 * F], dt.bfloat16, tag="bcx")
                nc.sync.dma_start(bcx2[:], bcx_d[cp])
                tbt2 = tpool.tile([n, 2 * FB], dt.bfloat16, tag="tbt")
                nc.sync.dma_start(tbt2[:], tbt_d[cp])
                tct2 = tpool.tile([n, 2 * FB], dt.bfloat16, tag="tct")
                nc.sync.dma_start(tct2[:], tct_d[cp])
                yout2 = opool.tile([L, 2 * FP], dt.float32, tag="yout")

                bcx_f32 = bcx2[:].bitcast(dt.float32)  # (L, F)

                for j in range(2):
                    gps = psG.tile([L, FB], dt.float32, tag="gps")
                    for i in range(nbh):
                        tb_i = tbt2[:, j * FB + i * L : j * FB + (i + 1) * L]
                        tc_i = tct2[:, j * FB + i * L : j * FB + (i + 1) * L]
                        nc.tensor.matmul(
                            gps[:, i * L : (i + 1) * L], tb_i, tc_i,
                            start=True, stop=True,
                        )

                    m1 = wpool.tile([L, FB], dt.bfloat16, tag="m1")
                    nc.vector.tensor_mul(m1[:], gps[:], maskx[:])

                    yps = psY.tile([L, FP], dt.float32, tag="yps")
                    sps = psS.tile([n, FP], dt.float32, tag="sps")
                    for i in range(nbh):
                        x_i = bcx2[:, j * F + i * BODY_F : j * F + i * BODY_F + p]
                        bt_i = bcx2[
                            :, j * F + i * BODY_F + p : j * F + i * BODY_F + 2 * p
                        ]
                        tc_i = tct2[:, j * FB + i * L : j * FB + (i + 1) * L]
                        ys = yps[:, i * p : (i + 1) * p]
                        nc.tensor.matmul(
                            ys, m1[:, i * L : (i + 1) * L], x_i, start=True, stop=False
                        )
                        nc.tensor.matmul(
                            ys, tc_i, rb_prev[:, i * p : (i + 1) * p],
                            start=False, stop=True,
                        )
                        nc.tensor.matmul(
                            sps[:, i * p : (i + 1) * p], bt_i, x_i,
                            start=True, stop=True,
                        )

                    nc.scalar.copy(yout2[:, j * FP : (j + 1) * FP], yps[:])

                    # R <- exp(tot) * (R + S); rb = bf16(R)
                    nc.vector.tensor_add(r_big[:], r_big[:], sps[:])
                    d_bc = (
                        bcx_f32[0:n, j * F // 2 : (j + 1) * F // 2]
                        .rearrange("q (i f) -> q i f", f=BODY_F // 2)[:, :, p : p + 1]
                        .broadcast_to((n, nbh, p))
                    )
                    r_3d = r_big[:].rearrange("q (i f) -> q i f", f=p)
                    nc.gpsimd.tensor_mul(r_3d, r_3d, d_bc)
                    rb = rbpool.tile([n, FP], dt.bfloat16, tag="rb")
                    nc.scalar.copy(rb[:], r_big[:])
                    rb_prev = rb

                nc.sync.dma_start(y_d[cp], yout2[:])

    nc.compile()
    return nc


def _host_prep(X, A, B, C, initial_states, hpc):
    """Build the packed/prescaled per-core input arrays."""
    b, s, h, p = X.shape
    n = B.shape[-1]
    c = s // L
    c2 = c // 2

    # within-chunk inclusive cumsum of A: (b, h, c, L)
    Ar = np.ascontiguousarray(A.reshape(b, c, L, h).transpose(0, 3, 1, 2))
    cum = np.cumsum(Ar, axis=-1, dtype=_f32)
    # align to (b, c, L, h) for broadcasting against B/C/X reshapes
    e_neg = np.exp(-cum).transpose(0, 2, 3, 1)[..., None]  # (b, c, L, h, 1)
    e_pos = np.exp(cum).transpose(0, 2, 3, 1)[..., None]
    d = np.exp(cum[..., -1])  # (b, h, c)

    Bt = (B.reshape(b, c, L, h, n) * e_neg).astype(_bf16)
    Ct = (C.reshape(b, c, L, h, n) * e_pos).astype(_bf16)

    # BCX: (c2, L, 2, b, h, BODY_F) -> flatten
    full = np.zeros((b, c, L, h, BODY_F), dtype=_bf16)
    full[..., 0:p] = X.reshape(b, c, L, h, p).astype(_bf16)
    full[..., p : p + n] = Bt
    daux = np.zeros((b, c, L, h, 2), dtype=_f32)
    daux[..., 0] = d.transpose(0, 2, 1)[:, :, None, :]  # (b, c, 1, h) broadcast over L
    full[..., p + n : p + n + 4] = daux.view(np.uint16).view(_bf16)
    bcx = np.ascontiguousarray(
        full.reshape(b, c2, 2, L, h, BODY_F).transpose(1, 3, 2, 0, 4, 5)
    )  # (c2, L, 2, b, h, BODY_F)

    # TBT/TCT: (c2, n, 2, b, h, L)
    tbt = np.ascontiguousarray(
        Bt.reshape(b, c2, 2, L, h, n).transpose(1, 5, 2, 0, 4, 3)
    )
    tct = np.ascontiguousarray(
        Ct.reshape(b, c2, 2, L, h, n).transpose(1, 5, 2, 0, 4, 3)
    )

    # INIT: (b, h, p, n) -> (n, b, h, p)
    init_t = np.ascontiguousarray(
        initial_states[:, 0].transpose(3, 0, 1, 2)
    ).astype(_f32)

    return bcx, tbt, tct, init_t


def kernel(X, A, B, C, initial_states):
    from concourse.bass_utils import run_bass_kernel_spmd

    X = np.asarray(X)
    A = np.asarray(A)
    B = np.asarray(B)
    C = np.asarray(C)
    initial_states = np.asarray(initial_states)

    b, s, h, p = X.shape
    n = B.shape[-1]
    assert h % N_CORES == 0, f"need h % {N_CORES} == 0, got h={h}"
    hpc = h // N_CORES
    c = s // L
    c2 = c // 2
    nbh = b * hpc

    bcx, tbt, tct, init_t = _host_prep(X, A, B, C, initial_states, hpc)

    mask = np.triu(np.ones((L, L), dtype=_f32)).astype(_bf16)
    maskx = np.ascontiguousarray(np.broadcast_to(mask[:, None, :], (L, nbh, L)))

    nc = _build_program(b, s, hpc, p, n)

    in_maps = []
    for k in range(N_CORES):
        hs = slice(k * hpc, (k + 1) * hpc)
        in_maps.append(
            {
                "BCX": np.ascontiguousarray(bcx[:, :, :, :, hs]).reshape(
                    c2, L, 2 * nbh * BODY_F
                ),
                "TBT": np.ascontiguousarray(tbt[:, :, :, :, hs]).reshape(
                    c2, n, 2 * nbh * L
                ),
                "TCT": np.ascontiguousarray(tct[:, :, :, :, hs]).reshape(
                    c2, n, 2 * nbh * L
                ),
                "INIT": np.ascontiguousarray(init_t[:, :, hs]).reshape(n, nbh * p),
                "MASKX": maskx.reshape(L, nbh * L),
            }
        )

    trace = _maybe_enable_tracing()
    kw = {}
    if trace:
        kw = dict(trace=True, tmpdir=os.environ.get("BASS_KERNEL_TRACE_DIR") or None)
    res = run_bass_kernel_spmd(nc, in_maps, list(range(N_CORES)), **kw)
    if trace and res.exec_time_ns is not None:
        print(f"HW exec time: {res.exec_time_ns} ns")

    # Y per core: (c2, L, 2, b, hpc, p) -> (b, s, hpc, p); concat heads
    ys = []
    for k in range(N_CORES):
        yk = res.results[k]["Y"].reshape(c2, L, 2, b, hpc, p)
        ys.append(np.transpose(yk, (3, 0, 2, 1, 4, 5)))  # (b, c2, 2, L, hpc, p)
    Y = np.concatenate(ys, axis=4).reshape(b, s, h, p)
    return np.ascontiguousarray(Y).astype(_f32)
